# revision 10
# baseline (speedup 1.0000x reference)
"""Trainium2 Bass kernel for nn_CrossMultiHeadAttention.

H=8 independent single-head attention modules over (L=1024, N=8, E=1024),
sharded one head per NeuronCore across 8 cores. Each core computes its
head's QKV projections, attention (returning the softmax weights as an
output), the per-head output projection, and its head's partial
contribution to the final output linear. The host sums the 8 partial
contributions (the unshard step for a head-sharded matmul), adds the
final bias, and restores layouts.

All matmuls run in bf16 with fp32 PSUM accumulation; the softmax
(exp / row-sum / normalize) is computed in fp32 from the fp32 scores.
"""

import os
import sys
import types

import numpy as np


L, N, E, H = 1024, 8, 1024, 8
P = 128
EC = E // P   # 8 chunks of the feature axis
SCALE = 1.0 / 32.0   # 1/sqrt(E)


# --------------------------------------------------------------------------
# Environment shim: register the NTFF profile hook module if absent so that
# concourse imports cleanly under axon (harmless when tracing is unused).
def _ensure_axon_hooks():
    if "antenv.axon_hooks" in sys.modules:
        return
    try:
        import antenv  # noqa: F401
        import antenv.axon_hooks  # noqa: F401
        return
    except ImportError:
        pass
    m = types.ModuleType("antenv.axon_hooks")
    m._hook = None
    m.set_axon_ntff_profile_hook = lambda h: setattr(m, "_hook", h)
    m.get_axon_ntff_profile_hook = lambda: m._hook
    sys.modules["antenv.axon_hooks"] = m
    try:
        import antenv
        antenv.axon_hooks = m
        from trn_agent_boot.trn_boot import _ntff_profile_via_ctypes
        m.set_axon_ntff_profile_hook(
            _ntff_profile_via_ctypes("/opt/axon/libaxon_pjrt.so"))
    except Exception:
        pass


_ensure_axon_hooks()

import ml_dtypes  # noqa: E402
import concourse.bass as bass  # noqa: E402
import concourse.tile as tile  # noqa: E402
from concourse import mybir  # noqa: E402
from concourse.bass_utils import run_bass_kernel_spmd  # noqa: E402

BF = mybir.dt.bfloat16
F32 = mybir.dt.float32
AF = mybir.ActivationFunctionType


# --------------------------------------------------------------------------
# The walrus build in this toolchain caps the number of sem waits one
# instruction may carry (DMA: 1, most others: 2; "Too many sync wait
# commands" otherwise). Tile doesn't know the cap, so split excess waits
# onto standalone EventSemaphore instructions on the same engine, placed
# immediately before the overloaded instruction.
def _wait_cap(inst) -> int:
    # This walrus build rejects >1 sem wait on most instruction structs.
    return 1


class FixedTileContext(tile.TileContext):
    def _add_instruction(self, inst):
        si = inst.sync_info
        if si is not None and si.on_wait:
            waits = list(si.on_wait)
            cap = _wait_cap(inst)
            if len(waits) > cap:
                keep = waits[-cap:]
                for w in waits[:-cap]:
                    ev = mybir.InstEventSemaphore(
                        name=self.nc.get_next_instruction_name(), ins=[], outs=[])
                    ev.engine = inst.engine
                    ev.sync_info = mybir.SyncInfo(on_wait=[w], on_update=[])
                    super()._add_instruction(ev)
                si.on_wait = keep
        super()._add_instruction(inst)

    def _drain_and_barrier(self, tick_clock, wait_clock):
        gc = tick_clock.global_clock
        alloc = wait_clock.sems.allocated()
        for proc, sem in sorted(alloc.items()):
            tick = gc[proc]
            if tick <= 0:
                continue
            mult = 16 if "DMA" in sem.name else 1
            self.nc.sync.wait_ge(sem, tick * mult)
        self.nc.sync.drain()
        self.nc.all_engine_barrier()
        popped = self.nc._tile_sem_poison_stack.pop()
        assert popped is self._sem_poison
        self.nc.clear_and_free_semaphores(list(self.sems.allocated().values()))
        self.nc.all_engine_barrier()


# --------------------------------------------------------------------------
def build_kernel(n_count: int = N):
    """Build the per-core SPMD program. One core <-> one head."""
    nc = bass.Bass()

    qT_d = nc.dram_tensor("qT", [E, N, L], BF, kind="ExternalInput")
    kT_d = nc.dram_tensor("kT", [E, N, L], BF, kind="ExternalInput")
    vT_d = nc.dram_tensor("vT", [E, N, L], BF, kind="ExternalInput")
    wq_d = nc.dram_tensor("wq", [E, E], BF, kind="ExternalInput")
    wk_d = nc.dram_tensor("wk", [E, E], BF, kind="ExternalInput")
    wv_d = nc.dram_tensor("wv", [E, E], BF, kind="ExternalInput")
    wo_d = nc.dram_tensor("wo", [E, E], BF, kind="ExternalInput")
    wu_d = nc.dram_tensor("wu", [E, E], BF, kind="ExternalInput")
    bq_d = nc.dram_tensor("bq", [P, EC], F32, kind="ExternalInput")
    bk_d = nc.dram_tensor("bk", [P, EC], F32, kind="ExternalInput")
    bvb_d = nc.dram_tensor("bvb", [P, E], F32, kind="ExternalInput")
    bo_d = nc.dram_tensor("bo", [P, EC], F32, kind="ExternalInput")

    attn_d = nc.dram_tensor("attn", [N, L, L], F32, kind="ExternalOutput")
    po_d = nc.dram_tensor("partialT", [E, N, L], F32, kind="ExternalOutput")

    with FixedTileContext(nc) as tc:
        with (
            tc.tile_pool(name="wres", bufs=1) as wres,      # resident weights
            tc.tile_pool(name="wstr", bufs=2) as wstr,      # streamed qkv weights
            tc.tile_pool(name="consts", bufs=1) as consts,
            tc.tile_pool(name="inp", bufs=2) as inp,
            tc.tile_pool(name="kqv", bufs=1) as kqv,
            tc.tile_pool(name="soft", bufs=2) as soft,
            tc.tile_pool(name="att", bufs=2) as att,
            tc.tile_pool(name="attT", bufs=1) as attTp,
            tc.tile_pool(name="ctx", bufs=2) as ctxp,
            tc.tile_pool(name="hop", bufs=2) as hop,
            tc.tile_pool(name="pop", bufs=2) as popp,
            tc.tile_pool(name="psA", bufs=4, space="PSUM") as psA,
            tc.tile_pool(name="psS", bufs=2, space="PSUM") as psS,
        ):
            # ---- resident weights / constants --------------------------------
            wo_sb = wres.tile([P, EC, E], BF, tag="wo")
            nc.sync.dma_start(wo_sb[:], wo_d.rearrange("(c p) e -> p c e", p=P))
            wu_sb = wres.tile([P, EC, E], BF, tag="wu")
            nc.sync.dma_start(wu_sb[:], wu_d.rearrange("(c p) e -> p c e", p=P))

            bq_sb = consts.tile([P, EC], F32, tag="bq")
            nc.sync.dma_start(bq_sb[:], bq_d[:])
            bk_sb = consts.tile([P, EC], F32, tag="bk")
            nc.sync.dma_start(bk_sb[:], bk_d[:])
            bvb_sb = consts.tile([P, E], F32, tag="bvb")
            nc.sync.dma_start(bvb_sb[:], bvb_d[:])
            bo_sb = consts.tile([P, EC], F32, tag="bo")
            nc.sync.dma_start(bo_sb[:], bo_d[:])

            for n in range(n_count):
                # ---- projections for batch n -----------------------------
                # kT_n[d, s], v_n[s, d], qT_n[d, l]; all bf16 in SBUF.
                kTn = kqv.tile([P, EC, L], BF, tag="kTn")
                vn = kqv.tile([P, EC, L], BF, tag="vn")
                qTn = kqv.tile([P, EC, L], BF, tag="qTn")

                for which in ("k", "v", "q"):
                    w_d = {"k": wk_d, "v": wv_d, "q": wq_d}[which]
                    src = {"k": kT_d, "v": vT_d, "q": qT_d}[which]
                    w_sb = wstr.tile([P, EC, E], BF, tag="w")
                    w_src = w_d.rearrange("(c p) d -> p c d", p=P)
                    if n == 0:
                        # fine-grained first loads so the PE can start on
                        # chunk ec as soon as it lands
                        for ec in range(EC):
                            nc.sync.dma_start(
                                w_sb[:, ec, :], w_src[:, ec, :])
                    else:
                        nc.sync.dma_start(w_sb[:], w_src)
                    for lb in range(2):
                        x_sb = inp.tile([P, EC, 512], BF, tag="x")
                        x_src = (src[:, n, lb * 512:(lb + 1) * 512]
                                 .rearrange("(c p) l -> p c l", p=P))
                        if n == 0:
                            for ec in range(EC):
                                nc.sync.dma_start(
                                    x_sb[:, ec, :], x_src[:, ec, :])
                        else:
                            nc.sync.dma_start(x_sb[:], x_src)
                        if which == "v":
                            # v_n[s, d]: activation chunk is stationary,
                            # weight chunk is the moving operand.
                            for st in range(4):
                                for db in range(2):
                                    ps = psA.tile([P, 512], F32, tag="psA")
                                    for ec in range(EC):
                                        nc.tensor.matmul(
                                            ps[:],
                                            x_sb[:, ec, st * P:(st + 1) * P],
                                            w_sb[:, ec,
                                                 db * 512:(db + 1) * 512],
                                            start=(ec == 0),
                                            stop=(ec == EC - 1))
                                    nc.vector.tensor_add(
                                        vn[:, lb * 4 + st,
                                           db * 512:(db + 1) * 512],
                                        ps[:],
                                        bvb_sb[:, db * 512:(db + 1) * 512])
                            continue
                        for dc in range(EC):
                            ps = psA.tile([P, 512], F32, tag="psA")
                            for ec in range(EC):
                                nc.tensor.matmul(
                                    ps[:],
                                    w_sb[:, ec, dc * P:(dc + 1) * P],
                                    x_sb[:, ec, :],
                                    start=(ec == 0), stop=(ec == EC - 1))
                            sl = slice(lb * 512, (lb + 1) * 512)
                            if which == "k":
                                nc.scalar.activation(
                                    kTn[:, dc, sl], ps[:], AF.Identity,
                                    bias=bk_sb[:, dc:dc + 1])
                            else:
                                nc.scalar.activation(
                                    qTn[:, dc, sl], ps[:], AF.Identity,
                                    bias=bq_sb[:, dc:dc + 1])

                # ---- attention for batch n -------------------------------
                for lb in range(2):
                    attnT = attTp.tile([P, EC, 512], BF, tag="attnT")
                    for lt in range(4):
                        l0 = lb * 512 + lt * P
                        ps_s = psS.tile([P, L], F32, tag="psS")
                        for sh in range(2):
                            for dc in range(EC):
                                nc.tensor.matmul(
                                    ps_s[:, sh * 512:(sh + 1) * 512],
                                    qTn[:, dc, l0:l0 + P],
                                    kTn[:, dc, sh * 512:(sh + 1) * 512],
                                    start=(dc == 0), stop=(dc == EC - 1))
                        exp_t = soft.tile([P, L], F32, tag="exp")
                        sum_t = soft.tile([P, 1], F32, tag="sum")
                        nc.scalar.activation(
                            exp_t[:], ps_s[:], AF.Exp, scale=SCALE,
                            accum_out=sum_t[:])
                        rec_t = soft.tile([P, 1], F32, tag="rec")
                        nc.vector.reciprocal(rec_t[:], sum_t[:])
                        attn_t = att.tile([P, L], F32, tag="attn")
                        nc.scalar.activation(
                            attn_t[:], exp_t[:], AF.Copy, scale=rec_t[:])
                        nc.sync.dma_start(
                            attn_d[n, l0:l0 + P, :], attn_t[:])
                        # bf16 copy of the weights for the ctx matmul,
                        # transposed on the DMA xbar (PE stays on matmuls)
                        attn_b = att.tile([P, L], BF, tag="attnb")
                        nc.vector.tensor_scalar_mul(
                            attn_b[:], exp_t[:], rec_t[:])
                        for sc in range(EC):
                            nc.sync.dma_start(
                                attnT[:, sc, lt * P:(lt + 1) * P],
                                attn_b[:, sc * P:(sc + 1) * P],
                                transpose=True)
                    # ctxT[d, l] for this l block
                    ctxT = ctxp.tile([P, EC, 512], BF, tag="ctxT")
                    for dc in range(EC):
                        ps = psA.tile([P, 512], F32, tag="psA")
                        for sc in range(EC):
                            nc.tensor.matmul(
                                ps[:],
                                vn[:, sc, dc * P:(dc + 1) * P],
                                attnT[:, sc, :],
                                start=(sc == 0), stop=(sc == EC - 1))
                        nc.vector.tensor_copy(ctxT[:, dc, :], ps[:])
                    # head_outT[e, l] = WoT.T @ ctxT + bo
                    ho = hop.tile([P, EC, 512], BF, tag="ho")
                    for ecb in range(EC):
                        ps = psA.tile([P, 512], F32, tag="psA")
                        for dc in range(EC):
                            nc.tensor.matmul(
                                ps[:],
                                wo_sb[:, dc, ecb * P:(ecb + 1) * P],
                                ctxT[:, dc, :],
                                start=(dc == 0), stop=(dc == EC - 1))
                        nc.scalar.activation(
                            ho[:, ecb, :], ps[:], AF.Identity,
                            bias=bo_sb[:, ecb:ecb + 1])
                    # partialT[eo, l] = WoutT.T @ ho  (bias added on host)
                    for oc in range(EC):
                        ps = psA.tile([P, 512], F32, tag="psA")
                        for ecb in range(EC):
                            nc.tensor.matmul(
                                ps[:],
                                wu_sb[:, ecb, oc * P:(oc + 1) * P],
                                ho[:, ecb, :],
                                start=(ecb == 0), stop=(ecb == EC - 1))
                        po_sb = popp.tile([P, 512], F32, tag="po")
                        nc.vector.tensor_copy(po_sb[:], ps[:])
                        nc.sync.dma_start(
                            po_d[oc * P:(oc + 1) * P, n,
                                 lb * 512:(lb + 1) * 512],
                            po_sb[:])
    return nc


_NC_CACHE = {}


def _get_nc(n_count=N):
    if n_count not in _NC_CACHE:
        _NC_CACHE[n_count] = build_kernel(n_count)
    return _NC_CACHE[n_count]


def kernel(query, key, value, Wqkv, bqkv, Wo, bo, Wout, bout,
           n_count: int = N, trace: bool = False):
    bf16 = ml_dtypes.bfloat16
    query = np.asarray(query)
    key = np.asarray(key)
    value = np.asarray(value)
    Wqkv = np.asarray(Wqkv)
    bqkv = np.asarray(bqkv)
    Wo = np.asarray(Wo)
    bo = np.asarray(bo)
    Wout = np.asarray(Wout)
    bout = np.asarray(bout)

    # Shared activations, feature-major: [E, N, L]
    qT = np.ascontiguousarray(query.transpose(2, 1, 0)).astype(bf16)
    kT = np.ascontiguousarray(key.transpose(2, 1, 0)).astype(bf16)
    vT = np.ascontiguousarray(value.transpose(2, 1, 0)).astype(bf16)

    in_maps = []
    for h in range(H):
        Wq = Wqkv[h, :E, :]
        Wk = Wqkv[h, E:2 * E, :]
        Wv = Wqkv[h, 2 * E:, :]
        in_maps.append({
            "qT": qT, "kT": kT, "vT": vT,
            "wq": np.ascontiguousarray(Wq.T).astype(bf16),
            "wk": np.ascontiguousarray(Wk.T).astype(bf16),
            "wv": np.ascontiguousarray(Wv.T).astype(bf16),
            "wo": np.ascontiguousarray(Wo[h].T).astype(bf16),
            "wu": np.ascontiguousarray(
                Wout[:, h * E:(h + 1) * E].T).astype(bf16),
            "bq": np.ascontiguousarray(
                bqkv[h, :E].reshape(EC, P).T).astype(np.float32),
            "bk": np.ascontiguousarray(
                bqkv[h, E:2 * E].reshape(EC, P).T).astype(np.float32),
            "bvb": np.ascontiguousarray(np.broadcast_to(
                bqkv[h, 2 * E:], (P, E))).astype(np.float32),
            "bo": np.ascontiguousarray(
                bo[h].reshape(EC, P).T).astype(np.float32),
        })

    nc = _get_nc(n_count)
    res = run_bass_kernel_spmd(nc, in_maps, list(range(H)), trace=trace)

    attn = np.stack([res.results[h]["attn"] for h in range(H)], axis=0)
    partial = np.zeros((E, N, L), np.float64)
    for h in range(H):
        partial += res.results[h]["partialT"].astype(np.float64)
    out = partial.transpose(2, 1, 0).astype(np.float32) + bout[None, None, :]
    if trace:
        kernel.last_exec_time_ns = res.exec_time_ns
        kernel.last_trace = res.instructions_and_trace
    return out, attn


# revision 12
# speedup vs baseline: 1.3674x; 1.3674x over previous
"""Trainium2 Bass kernel for nn_CrossMultiHeadAttention.

H=8 independent single-head attention modules over (L=1024, N=8, E=1024),
sharded one head per NeuronCore across 8 cores. Each core computes its
head's QKV projections, attention (returning the softmax weights as an
output), the per-head output projection, and its head's partial
contribution to the final output linear. The host sums the 8 partial
contributions (the unshard step for a head-sharded matmul), adds the
final bias, and restores layouts.

All matmuls run in bf16 with fp32 PSUM accumulation; the softmax
(exp / row-sum / normalize) is computed in fp32 from the fp32 scores.
"""

import os
import sys
import types

import numpy as np


L, N, E, H = 1024, 8, 1024, 8
P = 128
EC = E // P   # 8 chunks of the feature axis
SCALE = 1.0 / 32.0   # 1/sqrt(E)


# --------------------------------------------------------------------------
# Environment shim: register the NTFF profile hook module if absent so that
# concourse imports cleanly under axon (harmless when tracing is unused).
def _ensure_axon_hooks():
    if "antenv.axon_hooks" in sys.modules:
        return
    try:
        import antenv  # noqa: F401
        import antenv.axon_hooks  # noqa: F401
        return
    except ImportError:
        pass
    m = types.ModuleType("antenv.axon_hooks")
    m._hook = None
    m.set_axon_ntff_profile_hook = lambda h: setattr(m, "_hook", h)
    m.get_axon_ntff_profile_hook = lambda: m._hook
    sys.modules["antenv.axon_hooks"] = m
    try:
        import antenv
        antenv.axon_hooks = m
        from trn_agent_boot.trn_boot import _ntff_profile_via_ctypes
        m.set_axon_ntff_profile_hook(
            _ntff_profile_via_ctypes("/opt/axon/libaxon_pjrt.so"))
    except Exception:
        pass


_ensure_axon_hooks()

import ml_dtypes  # noqa: E402
import concourse.bass as bass  # noqa: E402
import concourse.tile as tile  # noqa: E402
from concourse import mybir  # noqa: E402
from concourse.bass_utils import run_bass_kernel_spmd  # noqa: E402

BF = mybir.dt.bfloat16
F32 = mybir.dt.float32
AF = mybir.ActivationFunctionType


# --------------------------------------------------------------------------
# The walrus build in this toolchain caps the number of sem waits one
# instruction may carry (DMA: 1, most others: 2; "Too many sync wait
# commands" otherwise). Tile doesn't know the cap, so split excess waits
# onto standalone EventSemaphore instructions on the same engine, placed
# immediately before the overloaded instruction.
def _wait_cap(inst) -> int:
    # This walrus build rejects >1 sem wait on most instruction structs.
    return 1


class FixedTileContext(tile.TileContext):
    def _add_instruction(self, inst):
        si = inst.sync_info
        if si is not None and si.on_wait:
            waits = list(si.on_wait)
            cap = _wait_cap(inst)
            if len(waits) > cap:
                keep = waits[-cap:]
                for w in waits[:-cap]:
                    ev = mybir.InstEventSemaphore(
                        name=self.nc.get_next_instruction_name(), ins=[], outs=[])
                    ev.engine = inst.engine
                    ev.sync_info = mybir.SyncInfo(on_wait=[w], on_update=[])
                    super()._add_instruction(ev)
                si.on_wait = keep
        super()._add_instruction(inst)

    def _drain_and_barrier(self, tick_clock, wait_clock):
        gc = tick_clock.global_clock
        alloc = wait_clock.sems.allocated()
        for proc, sem in sorted(alloc.items()):
            tick = gc[proc]
            if tick <= 0:
                continue
            mult = 16 if "DMA" in sem.name else 1
            self.nc.sync.wait_ge(sem, tick * mult)
        self.nc.sync.drain()
        self.nc.all_engine_barrier()
        popped = self.nc._tile_sem_poison_stack.pop()
        assert popped is self._sem_poison
        self.nc.clear_and_free_semaphores(list(self.sems.allocated().values()))
        self.nc.all_engine_barrier()


# --------------------------------------------------------------------------
def build_kernel(n_count: int = N):
    """Build the per-core SPMD program. One core <-> one head."""
    nc = bass.Bass()

    qT_d = nc.dram_tensor("qT", [E, N, L], BF, kind="ExternalInput")
    kT_d = nc.dram_tensor("kT", [E, N, L], BF, kind="ExternalInput")
    vT_d = nc.dram_tensor("vT", [E, N, L], BF, kind="ExternalInput")
    wq_d = nc.dram_tensor("wq", [E, E], BF, kind="ExternalInput")
    wk_d = nc.dram_tensor("wk", [E, E], BF, kind="ExternalInput")
    wv_d = nc.dram_tensor("wv", [E, E], BF, kind="ExternalInput")
    wo_d = nc.dram_tensor("wo", [E, E], BF, kind="ExternalInput")
    wu_d = nc.dram_tensor("wu", [E, E], BF, kind="ExternalInput")
    bq_d = nc.dram_tensor("bq", [P, EC], F32, kind="ExternalInput")
    bk_d = nc.dram_tensor("bk", [P, EC], F32, kind="ExternalInput")
    bvb_d = nc.dram_tensor("bvb", [P, E], F32, kind="ExternalInput")
    bo_d = nc.dram_tensor("bo", [P, EC], F32, kind="ExternalInput")
    id_d = nc.dram_tensor("ident", [P, P], BF, kind="ExternalInput")

    attn_d = nc.dram_tensor("attn", [N, L, L], F32, kind="ExternalOutput")
    po_d = nc.dram_tensor("partialT", [E, N, L], F32, kind="ExternalOutput")

    with FixedTileContext(nc) as tc:
        with (
            tc.tile_pool(name="wres", bufs=1) as wres,      # resident weights
            tc.tile_pool(name="wstr", bufs=2) as wstr,      # streamed qkv weights
            tc.tile_pool(name="consts", bufs=1) as consts,
            tc.tile_pool(name="inp", bufs=2) as inp,
            tc.tile_pool(name="kqv", bufs=1) as kqv,
            tc.tile_pool(name="soft", bufs=2) as soft,
            tc.tile_pool(name="att", bufs=2) as att,
            tc.tile_pool(name="attT", bufs=1) as attTp,
            tc.tile_pool(name="ctx", bufs=2) as ctxp,
            tc.tile_pool(name="hop", bufs=2) as hop,
            tc.tile_pool(name="pop", bufs=2) as popp,
            tc.tile_pool(name="psA", bufs=2, space="PSUM") as psA,
            tc.tile_pool(name="psS", bufs=2, space="PSUM") as psS,
            tc.tile_pool(name="psT", bufs=2, space="PSUM") as psT,
        ):
            # ---- resident weights / constants --------------------------------
            wo_sb = wres.tile([P, EC, E], BF, tag="wo")
            nc.sync.dma_start(wo_sb[:], wo_d.rearrange("(c p) e -> p c e", p=P))
            wu_sb = wres.tile([P, EC, E], BF, tag="wu")
            nc.sync.dma_start(wu_sb[:], wu_d.rearrange("(c p) e -> p c e", p=P))

            bq_sb = consts.tile([P, EC], F32, tag="bq")
            nc.sync.dma_start(bq_sb[:], bq_d[:])
            bk_sb = consts.tile([P, EC], F32, tag="bk")
            nc.sync.dma_start(bk_sb[:], bk_d[:])
            bvb_sb = consts.tile([P, E], F32, tag="bvb")
            nc.sync.dma_start(bvb_sb[:], bvb_d[:])
            bo_sb = consts.tile([P, EC], F32, tag="bo")
            nc.sync.dma_start(bo_sb[:], bo_d[:])
            id_sb = consts.tile([P, P], BF, tag="ident")
            nc.sync.dma_start(id_sb[:], id_d[:])

            for n in range(n_count):
                # ---- projections for batch n -----------------------------
                # kT_n[d, s], v_n[s, d], qT_n[d, l]; all bf16 in SBUF.
                kTn = kqv.tile([P, EC, L], BF, tag="kTn")
                vn = kqv.tile([P, EC, L], BF, tag="vn")
                qTn = kqv.tile([P, EC, L], BF, tag="qTn")

                for which in ("k", "v", "q"):
                    w_d = {"k": wk_d, "v": wv_d, "q": wq_d}[which]
                    src = {"k": kT_d, "v": vT_d, "q": qT_d}[which]
                    w_sb = wstr.tile([P, EC, E], BF, tag="w")
                    w_src = w_d.rearrange("(c p) d -> p c d", p=P)
                    if n == 0:
                        # fine-grained first loads so the PE can start on
                        # chunk ec as soon as it lands
                        for ec in range(EC):
                            nc.sync.dma_start(
                                w_sb[:, ec, :], w_src[:, ec, :])
                    else:
                        nc.sync.dma_start(w_sb[:], w_src)
                    for lb in range(2):
                        x_sb = inp.tile([P, EC, 512], BF, tag="x")
                        x_src = (src[:, n, lb * 512:(lb + 1) * 512]
                                 .rearrange("(c p) l -> p c l", p=P))
                        if n == 0:
                            for ec in range(EC):
                                nc.sync.dma_start(
                                    x_sb[:, ec, :], x_src[:, ec, :])
                        else:
                            nc.sync.dma_start(x_sb[:], x_src)
                        if which == "v":
                            # v_n[s, d]: activation chunk is stationary,
                            # weight chunk is the moving operand.
                            for st in range(4):
                                for db in range(2):
                                    ps = psA.tile([P, 512], F32, tag="psA")
                                    for ec in range(EC):
                                        nc.tensor.matmul(
                                            ps[:],
                                            x_sb[:, ec, st * P:(st + 1) * P],
                                            w_sb[:, ec,
                                                 db * 512:(db + 1) * 512],
                                            start=(ec == 0),
                                            stop=(ec == EC - 1))
                                    nc.vector.tensor_add(
                                        vn[:, lb * 4 + st,
                                           db * 512:(db + 1) * 512],
                                        ps[:],
                                        bvb_sb[:, db * 512:(db + 1) * 512])
                            continue
                        for dc in range(EC):
                            ps = psA.tile([P, 512], F32, tag="psA")
                            for ec in range(EC):
                                nc.tensor.matmul(
                                    ps[:],
                                    w_sb[:, ec, dc * P:(dc + 1) * P],
                                    x_sb[:, ec, :],
                                    start=(ec == 0), stop=(ec == EC - 1))
                            sl = slice(lb * 512, (lb + 1) * 512)
                            if which == "k":
                                nc.scalar.activation(
                                    kTn[:, dc, sl], ps[:], AF.Identity,
                                    bias=bk_sb[:, dc:dc + 1])
                            else:
                                nc.scalar.activation(
                                    qTn[:, dc, sl], ps[:], AF.Identity,
                                    bias=bq_sb[:, dc:dc + 1])

                # ---- attention for batch n -------------------------------
                for lb in range(2):
                    attnT = attTp.tile([P, EC, 512], BF, tag="attnT")
                    for lt in range(4):
                        l0 = lb * 512 + lt * P
                        ps_s = psS.tile([P, L], F32, tag="psS")
                        for sh in range(2):
                            for dc in range(EC):
                                nc.tensor.matmul(
                                    ps_s[:, sh * 512:(sh + 1) * 512],
                                    qTn[:, dc, l0:l0 + P],
                                    kTn[:, dc, sh * 512:(sh + 1) * 512],
                                    start=(dc == 0), stop=(dc == EC - 1))
                        exp_t = soft.tile([P, L], F32, tag="exp")
                        sum_t = soft.tile([P, 1], F32, tag="sum")
                        nc.scalar.activation(
                            exp_t[:], ps_s[:], AF.Exp, scale=SCALE,
                            accum_out=sum_t[:])
                        rec_t = soft.tile([P, 1], F32, tag="rec")
                        nc.vector.reciprocal(rec_t[:], sum_t[:])
                        attn_t = att.tile([P, L], F32, tag="attn")
                        nc.scalar.activation(
                            attn_t[:], exp_t[:], AF.Copy, scale=rec_t[:])
                        nc.sync.dma_start(
                            attn_d[n, l0:l0 + P, :], attn_t[:])
                        # bf16 copy of the weights for the ctx matmul;
                        # transposed on the PE (bf16: 1 cycle/row)
                        attn_b = att.tile([P, L], BF, tag="attnb")
                        nc.vector.tensor_scalar_mul(
                            attn_b[:], exp_t[:], rec_t[:])
                        for sc in range(EC):
                            ps_t = psT.tile([P, P], BF, tag="psT")
                            nc.tensor.transpose(
                                ps_t[:], attn_b[:, sc * P:(sc + 1) * P],
                                id_sb[:])
                            nc.vector.tensor_copy(
                                attnT[:, sc, lt * P:(lt + 1) * P], ps_t[:])
                    # ctxT[d, l] for this l block
                    ctxT = ctxp.tile([P, EC, 512], BF, tag="ctxT")
                    for dc in range(EC):
                        ps = psA.tile([P, 512], F32, tag="psA")
                        for sc in range(EC):
                            nc.tensor.matmul(
                                ps[:],
                                vn[:, sc, dc * P:(dc + 1) * P],
                                attnT[:, sc, :],
                                start=(sc == 0), stop=(sc == EC - 1))
                        nc.vector.tensor_copy(ctxT[:, dc, :], ps[:])
                    # head_outT[e, l] = WoT.T @ ctxT + bo
                    ho = hop.tile([P, EC, 512], BF, tag="ho")
                    for ecb in range(EC):
                        ps = psA.tile([P, 512], F32, tag="psA")
                        for dc in range(EC):
                            nc.tensor.matmul(
                                ps[:],
                                wo_sb[:, dc, ecb * P:(ecb + 1) * P],
                                ctxT[:, dc, :],
                                start=(dc == 0), stop=(dc == EC - 1))
                        nc.scalar.activation(
                            ho[:, ecb, :], ps[:], AF.Identity,
                            bias=bo_sb[:, ecb:ecb + 1])
                    # partialT[eo, l] = WoutT.T @ ho  (bias added on host)
                    for oc in range(EC):
                        ps = psA.tile([P, 512], F32, tag="psA")
                        for ecb in range(EC):
                            nc.tensor.matmul(
                                ps[:],
                                wu_sb[:, ecb, oc * P:(oc + 1) * P],
                                ho[:, ecb, :],
                                start=(ecb == 0), stop=(ecb == EC - 1))
                        po_sb = popp.tile([P, 512], F32, tag="po")
                        nc.vector.tensor_copy(po_sb[:], ps[:])
                        nc.sync.dma_start(
                            po_d[oc * P:(oc + 1) * P, n,
                                 lb * 512:(lb + 1) * 512],
                            po_sb[:])
    return nc


_NC_CACHE = {}


def _get_nc(n_count=N):
    if n_count not in _NC_CACHE:
        _NC_CACHE[n_count] = build_kernel(n_count)
    return _NC_CACHE[n_count]


def kernel(query, key, value, Wqkv, bqkv, Wo, bo, Wout, bout,
           n_count: int = N, trace: bool = False):
    bf16 = ml_dtypes.bfloat16
    query = np.asarray(query)
    key = np.asarray(key)
    value = np.asarray(value)
    Wqkv = np.asarray(Wqkv)
    bqkv = np.asarray(bqkv)
    Wo = np.asarray(Wo)
    bo = np.asarray(bo)
    Wout = np.asarray(Wout)
    bout = np.asarray(bout)

    # Shared activations, feature-major: [E, N, L]
    qT = np.ascontiguousarray(query.transpose(2, 1, 0)).astype(bf16)
    kT = np.ascontiguousarray(key.transpose(2, 1, 0)).astype(bf16)
    vT = np.ascontiguousarray(value.transpose(2, 1, 0)).astype(bf16)
    ident = np.eye(P, dtype=np.float32).astype(bf16)

    in_maps = []
    for h in range(H):
        Wq = Wqkv[h, :E, :]
        Wk = Wqkv[h, E:2 * E, :]
        Wv = Wqkv[h, 2 * E:, :]
        in_maps.append({
            "qT": qT, "kT": kT, "vT": vT,
            "wq": np.ascontiguousarray(Wq.T).astype(bf16),
            "wk": np.ascontiguousarray(Wk.T).astype(bf16),
            "wv": np.ascontiguousarray(Wv.T).astype(bf16),
            "wo": np.ascontiguousarray(Wo[h].T).astype(bf16),
            "wu": np.ascontiguousarray(
                Wout[:, h * E:(h + 1) * E].T).astype(bf16),
            "bq": np.ascontiguousarray(
                bqkv[h, :E].reshape(EC, P).T).astype(np.float32),
            "bk": np.ascontiguousarray(
                bqkv[h, E:2 * E].reshape(EC, P).T).astype(np.float32),
            "bvb": np.ascontiguousarray(np.broadcast_to(
                bqkv[h, 2 * E:], (P, E))).astype(np.float32),
            "bo": np.ascontiguousarray(
                bo[h].reshape(EC, P).T).astype(np.float32),
            "ident": ident,
        })

    nc = _get_nc(n_count)
    res = run_bass_kernel_spmd(nc, in_maps, list(range(H)), trace=trace)

    attn = np.stack([res.results[h]["attn"] for h in range(H)], axis=0)
    partial = np.zeros((E, N, L), np.float64)
    for h in range(H):
        partial += res.results[h]["partialT"].astype(np.float64)
    out = partial.transpose(2, 1, 0).astype(np.float32) + bout[None, None, :]
    if trace:
        kernel.last_exec_time_ns = res.exec_time_ns
        kernel.last_trace = res.instructions_and_trace
    return out, attn
